# revision 19
# baseline (speedup 1.0000x reference)
"""Multi-head self-attention (B=2, N=2048, D=1024, H=16) on 8 Trainium2 cores.

Sharding: core c -> batch b = c // 4, head group g = c % 4 (heads 4g..4g+3).
Each core computes q/k/v for its 4 heads (bf16), attention with scores^T in
[j, i] layout, then PV in the "e-as-weights" form: for each 128-wide i-chunk,
matmul(lhsT=e[j, i-chunk], rhs=[v | ones]) accumulates out[i, d] and the
softmax denominator (65th column) in PSUM across the 16 j-chunks.  Normalize
is a per-partition reciprocal broadcast on DVE, transposed back to [d, i] via
an identity matmul into spare PSUM columns, and projected with both head
pairs accumulated in one PSUM group (K=256).  Host sums the 4 per-core
partial projections (+ residual x) per batch.
"""

import numpy as np
import ml_dtypes

import concourse.bass as bass
import concourse.bacc as bacc
import concourse.mybir as mybir
import concourse.tile as tile
from concourse.bass_utils import run_bass_kernel_spmd

B = 2
N = 2048
D = 1024
NH = 16
DH = 64
N_CORES = 8
TP = 4                # head-parallel ways per batch
HPC = NH // TP        # heads per core
HDIM = HPC * DH       # 256 head dims per core
PAIRS = HPC // 2
SCALE = 1.0 / 8.0     # 1/sqrt(DH)

IT = N // 512         # 4 i-tiles
JT = N // 128         # 16 j-chunks
KC = D // 128         # 8 feature chunks

F32 = mybir.dt.float32
BF16 = mybir.dt.bfloat16
F8 = mybir.dt.float8e4
NP_BF16 = ml_dtypes.bfloat16
NP_F8 = ml_dtypes.float8_e4m3
WSCALE = 64.0          # host scales w_qkv/w_proj by this; exp scale and the
                       # host-side combine divide it back out
SCALE_DEV = SCALE / (WSCALE * WSCALE)   # exactly 2**-15
AF = mybir.ActivationFunctionType


def build_bass():
    nc = bacc.Bacc("TRN2", target_bir_lowering=False, debug=False)
    xT = nc.declare_dram_parameter("xT", [D, N], F8, isOutput=False)
    # weights arrive pre-arranged in the on-chip layout: [128, KC/2, 2, HDIM]
    # (DoubleRow pair planes) flattened to 2KB rows for full-speed DMA
    wq = nc.declare_dram_parameter("wq", [128, D * HDIM // 128], F8, isOutput=False)
    wk = nc.declare_dram_parameter("wk", [128, D * HDIM // 128], F8, isOutput=False)
    wv = nc.declare_dram_parameter("wv", [128, D * HDIM // 128], F8, isOutput=False)
    wp = nc.declare_dram_parameter("wp", [128, PAIRS * D], F8, isOutput=False)
    ident_d = nc.declare_dram_parameter("ident_c", [128, 128], BF16, isOutput=False)
    pT = nc.declare_dram_parameter("pT", [D, N], F32, isOutput=True)

    with tile.TileContext(nc) as tc:
        with (
            tc.tile_pool(name="big", bufs=1) as big,
            tc.tile_pool(name="exps", bufs=6) as exps,
            tc.tile_pool(name="evac", bufs=4) as evac,
            tc.tile_pool(name="psum", bufs=1, space="PSUM") as psum,
        ):
            # ---- input loads: x chunks + wk first (kT p0 is the critical
            # path to the first scores), then wq, then the rest.
            xts = big.tile([128, KC, N], F8, tag="xts")
            xT_r = xT.rearrange("(c p) n -> p c n", p=128)
            wk_sb = big.tile([128, KC // 2, 2, HDIM], F8, tag="wk")
            wq_sb = big.tile([128, KC // 2, 2, HDIM], F8, tag="wq")
            wv_sb = big.tile([128, KC // 2, 2, HDIM], F8, tag="wv")
            wp_sb = big.tile([128, PAIRS, D], F8, tag="wp")
            ident = big.tile([128, 128], BF16, tag="ident")

            nc.sync.dma_start(out=xts[:, 0:2, :], in_=xT_r[:, 0:2, :])
            nc.scalar.dma_start(
                out=wk_sb, in_=wk.rearrange("p (c two w) -> p c two w", two=2, w=HDIM)
            )
            nc.scalar.dma_start(out=xts[:, 2:4, :], in_=xT_r[:, 2:4, :])
            nc.scalar.dma_start(
                out=wq_sb, in_=wq.rearrange("p (c two w) -> p c two w", two=2, w=HDIM)
            )
            nc.sync.dma_start(out=xts[:, 4:6, :], in_=xT_r[:, 4:6, :])
            nc.scalar.dma_start(
                out=wv_sb, in_=wv.rearrange("p (c two w) -> p c two w", two=2, w=HDIM)
            )
            nc.sync.dma_start(out=xts[:, 6:8, :], in_=xT_r[:, 6:8, :])
            nc.sync.dma_start(
                out=wp_sb, in_=wp.rearrange("p (c d) -> p c d", c=PAIRS)
            )
            nc.sync.dma_start(out=ident, in_=ident_d[:, :])

            # v_aug[j, jt, head, 0:64] = v, [..., 64] = 1.0 (denominator trick)
            v_aug = big.tile([128, JT, HPC, 65], BF16, tag="v")
            nc.vector.memset(v_aug[:, :, :, 64:65], 1.0)

            e_set0 = big.tile([128, JT, 1024], BF16, tag="eset0")
            e_set1 = big.tile([128, JT, 1024], BF16, tag="eset1")
            e_sets = [e_set0, e_set1]
            qT = big.tile([128, PAIRS, N], BF16, tag="qT")
            kT = big.tile([128, PAIRS, N], BF16, tag="kT")
            outT = big.tile([128, PAIRS, N], F8, tag="outT")

            warm = evac.tile([1, 1], F32, tag="warm")
            nc.scalar.activation(warm, ident[0:1, 0:1], AF.Exp)

            def emit_qk_tile(p, w_sb, dst, nt):
                ps = psum.tile([128, 512], F32, tag="mm", bufs=2)
                for c in range(KC // 2):
                    nc.tensor.matmul(
                        ps,
                        lhsT=w_sb[:, c, :, p * 128:(p + 1) * 128],
                        rhs=xts[:, 2 * c:2 * c + 2, nt * 512:(nt + 1) * 512],
                        start=(c == 0),
                        stop=(c == KC // 2 - 1),
                        perf_mode=mybir.MatmulPerfMode.DoubleRow,
                    )
                nc.vector.tensor_copy(dst[:, p, nt * 512:(nt + 1) * 512], ps)

            def emit_v_chunk(t):
                ps = psum.tile([128, 512], F32, tag="mm", bufs=2)
                for c in range(KC // 2):
                    nc.tensor.matmul(
                        ps[:, 0:HDIM],
                        lhsT=xts[:, 2 * c:2 * c + 2, t * 128:(t + 1) * 128],
                        rhs=wv_sb[:, c, :, :],
                        start=(c == 0),
                        stop=(c == KC // 2 - 1),
                        perf_mode=mybir.MatmulPerfMode.DoubleRow,
                    )
                nc.vector.tensor_copy(
                    v_aug[:, t, :, 0:64],
                    ps[:, 0:HDIM].rearrange("p (h c) -> p h c", c=64),
                )

            def emit_scores(p, it, jt):
                sc = psum.tile([128, 1024], F32, tag="sc", bufs=2)
                for h in range(2):
                    nc.tensor.matmul(
                        sc[:, h * 512:(h + 1) * 512],
                        lhsT=kT[h * 64:(h + 1) * 64, p, jt * 128:(jt + 1) * 128],
                        rhs=qT[h * 64:(h + 1) * 64, p, it * 512:(it + 1) * 512],
                        start=True,
                        stop=True,
                    )
                return sc

            def emit_pv_group(p, g, eset, pvt):
                # one accumulation group (h, ic): 16 back-to-back matmuls over
                # the j-chunks -- exactly one open group per PSUM bank.
                h, ic = divmod(g, 4)
                for jt in range(JT):
                    nc.tensor.matmul(
                        pvt[:, h * 512 + ic * 65:h * 512 + ic * 65 + 65],
                        lhsT=eset[:, jt, h * 512 + ic * 128:h * 512 + (ic + 1) * 128],
                        rhs=v_aug[:, jt, 2 * p + h, :],
                        start=(jt == 0),
                        stop=(jt == JT - 1),
                    )

            def emit_norm_transpose(p, it, pvt):
                # All pvt readers (recip + muls) come first so the next
                # window's PV (WAR on the single pv buffer) unblocks early;
                # transposes land in mm-pool scratch, then evac to outT.
                rc = evac.tile([128, 8], F32, tag="rc", bufs=2)
                dg = evac.tile([128, 8], F32, tag="dg", bufs=2)
                nc.vector.tensor_copy(
                    dg[:, 0:4],
                    pvt[:, 0:260].rearrange("p (g c) -> p g c", c=65)[:, :, 64],
                )
                nc.vector.tensor_copy(
                    dg[:, 4:8],
                    pvt[:, 512:772].rearrange("p (g c) -> p g c", c=65)[:, :, 64],
                )
                nc.vector.reciprocal(rc, dg)
                ois = []
                for ic in range(4):
                    oi = evac.tile([128, 128], BF16, tag="outI", bufs=4)
                    nc.vector.tensor_scalar_mul(
                        oi[:, 0:64],
                        pvt[:, ic * 65:ic * 65 + 64],
                        rc[:, ic:ic + 1],
                    )
                    nc.vector.tensor_scalar_mul(
                        oi[:, 64:128],
                        pvt[:, 512 + ic * 65:512 + ic * 65 + 64],
                        rc[:, 4 + ic:4 + ic + 1],
                    )
                    ois.append(oi)
                ps = psum.tile([128, 512], F32, tag="mm", bufs=2)
                for ic in range(4):
                    nc.tensor.matmul(
                        ps[:, ic * 128:(ic + 1) * 128],
                        lhsT=ois[ic], rhs=ident, start=True, stop=True,
                    )
                for ic in range(4):
                    nc.vector.tensor_copy(
                        outT[:, p, it * 512 + ic * 128:it * 512 + (ic + 1) * 128],
                        ps[:, ic * 128:(ic + 1) * 128],
                    )

            def emit_proj_tile(it, ot, slot=None):
                if slot is not None:
                    pj = slot
                else:
                    pj = psum.tile([128, 512], F32, tag="mm", bufs=2)
                nc.tensor.matmul(
                    pj,
                    lhsT=wp_sb[:, :, ot * 128:(ot + 1) * 128],
                    rhs=outT[:, :, it * 512:(it + 1) * 512],
                    start=True,
                    stop=True,
                    perf_mode=mybir.MatmulPerfMode.DoubleRow,
                )
                o_sb = evac.tile([128, 512], F32, tag="osb", bufs=8)
                if slot is not None and ot % 2 == 1:
                    nc.scalar.copy(o_sb, pj)
                else:
                    nc.vector.tensor_copy(o_sb, pj)
                deng = nc.scalar if (slot is not None and ot % 2 == 0) else nc.sync
                deng.dma_start(
                    out=pT[ot * 128:(ot + 1) * 128, it * 512:(it + 1) * 512],
                    in_=o_sb,
                )

            # ---- prelude: kT p0 chunk for jt 0-3, qT p0 it0 ----
            emit_qk_tile(0, wk_sb, kT, 0)
            emit_qk_tile(0, wq_sb, qT, 0)

            # filler work queue: (kind, pair, arg), ordered by first use.
            fillers = [
                ("k", 0, 1), ("q", 0, 1), ("k", 0, 2), ("k", 0, 3),
                ("k", 1, 0), ("q", 0, 2), ("k", 1, 1), ("q", 1, 0),
                ("k", 1, 2), ("k", 1, 3), ("q", 0, 3), ("q", 1, 1),
                ("q", 1, 2), ("q", 1, 3),
            ]
            proj_q = []

            def pop_filler():
                if fillers:
                    kind, p_, arg = fillers.pop(0)
                    emit_qk_tile(p_, wk_sb if kind == "k" else wq_sb,
                                 kT if kind == "k" else qT, arg)
                    return True
                if proj_q:
                    it_t, ot = proj_q.pop(0)
                    emit_proj_tile(it_t, ot)
                    return True
                return False

            # Flattened step stream over all (pair, i-tile, j-chunk) steps.
            # Scores are emitted one step ahead (across window boundaries).
            # exp(w, jt) fills e_sets[w % 2]; the PV of window w-1 runs as 8
            # sequential accumulation groups (one per PSUM bank at a time)
            # spread over the first steps of window w, followed by its
            # normalize/transpose chain and projection availability.
            windows = [(p, it) for p in range(PAIRS) for it in range(IT)]
            NW = len(windows)
            steps = NW * JT

            def step_scores(s):
                w, jt = divmod(s, JT)
                p, it = windows[w]
                return emit_scores(p, it, jt)

            prev = None          # (p, it, pvt, eset) of the previous window
            pvt = None
            sc_cur = step_scores(0)
            for s in range(steps):
                w, jt = divmod(s, JT)
                p, it = windows[w]
                eset = e_sets[w % 2]
                if jt == 0:
                    pvt = psum.tile([128, 1024], F32, tag="pv", bufs=1)
                sc = sc_cur
                sc_cur = step_scores(s + 1) if s + 1 < steps else None
                # fillers / v chunks
                if w == 0:
                    emit_v_chunk(jt)
                    if jt in (2, 4, 6, 10):
                        pop_filler()
                elif jt % 2 == 1:
                    pop_filler()
                    if proj_q and jt % 4 == 1:
                        pop_filler()
                nc.scalar.activation(eset[:, jt, :], sc, AF.Exp, scale=SCALE_DEV)
                if prev is not None:
                    if jt < 8:
                        emit_pv_group(prev[0], jt, prev[3], prev[2])
                    elif jt == 8:
                        p0, it0, pvt0, _ = prev
                        emit_norm_transpose(p0, it0, pvt0)
                        if p0 == 1:
                            proj_q.extend((it0, ot) for ot in range(D // 128))
                if jt == JT - 1:
                    prev = (p, it, pvt, eset)

            # ---- drain: last window's remaining PV groups + per-ic
            # normalize pipeline + projection (ACT helps with evacs) ----
            p0, it0, pvt0, eset0 = prev
            while fillers:
                pop_filler()
            tmm = psum.tile([128, 512], F32, tag="mm", bufs=2)

            def norm_ic(ic):
                rc = evac.tile([128, 2], F32, tag="rcd", bufs=4)
                nc.vector.reciprocal(
                    rc[:, 0:1], pvt0[:, ic * 65 + 64:ic * 65 + 65]
                )
                nc.vector.reciprocal(
                    rc[:, 1:2], pvt0[:, 512 + ic * 65 + 64:512 + ic * 65 + 65]
                )
                oi = evac.tile([128, 128], BF16, tag="outI", bufs=4)
                nc.vector.tensor_scalar_mul(
                    oi[:, 0:64], pvt0[:, ic * 65:ic * 65 + 64], rc[:, 0:1]
                )
                nc.vector.tensor_scalar_mul(
                    oi[:, 64:128],
                    pvt0[:, 512 + ic * 65:512 + ic * 65 + 64], rc[:, 1:2]
                )
                nc.tensor.matmul(
                    tmm[:, ic * 128:(ic + 1) * 128],
                    lhsT=oi, rhs=ident, start=True, stop=True,
                )
                nc.vector.tensor_copy(
                    outT[:, p0, it0 * 512 + ic * 128:it0 * 512 + (ic + 1) * 128],
                    tmm[:, ic * 128:(ic + 1) * 128],
                )

            # interleave so norm(ic)'s DVE chain hides under group ic+1's
            # matmul burst (the PE transpose then finds its input ready)
            emit_pv_group(p0, 0, eset0, pvt0)
            emit_pv_group(p0, 4, eset0, pvt0)
            for ic in range(4):
                if ic + 1 < 4:
                    emit_pv_group(p0, ic + 1, eset0, pvt0)
                    emit_pv_group(p0, 4 + ic + 1, eset0, pvt0)
                norm_ic(ic)

            drain_slots = []
            for _ in range(2):
                sct = psum.tile([128, 1024], F32, tag="sc", bufs=2)
                drain_slots += [sct[:, 0:512], sct[:, 512:1024]]
            proj_q.extend((it0, ot) for ot in range(D // 128))
            for di, (it_t, ot) in enumerate(proj_q):
                emit_proj_tile(it_t, ot, drain_slots[di % len(drain_slots)])
            proj_q = []
    return nc


_NC = None


def _get_nc():
    global _NC
    if _NC is None:
        _NC = build_bass()
        _NC.finalize()
    return _NC


_IDENT = np.eye(128, dtype=NP_BF16)


def _w_dr_layout(w):
    # [D, HDIM] -> [128, KC/2 * 2 * HDIM]: row p, flat (c, plane, m) with
    # plane = DoubleRow pair index; element = w[(2c + plane)*128 + p, m]
    r = w.reshape(KC // 2, 2, 128, HDIM).transpose(2, 0, 1, 3)
    return np.ascontiguousarray(r.reshape(128, -1))


def make_in_maps(x, w_qkv, w_proj):
    x = np.asarray(x, np.float32)
    w_qkv = (np.asarray(w_qkv, np.float32) * WSCALE).astype(NP_F8)
    w_proj = (np.asarray(w_proj, np.float32) * WSCALE).astype(NP_F8)
    xTs = [np.ascontiguousarray(x[b].T.astype(NP_F8)) for b in range(B)]
    in_maps = []
    for c in range(N_CORES):
        b, g = divmod(c, TP)
        h0 = g * HDIM
        wp_r = w_proj[h0:h0 + HDIM, :].reshape(PAIRS, 128, D).transpose(1, 0, 2)
        in_maps.append({
            "xT": xTs[b],
            "wq": _w_dr_layout(w_qkv[:, h0:h0 + HDIM]),
            "wk": _w_dr_layout(w_qkv[:, D + h0:D + h0 + HDIM]),
            "wv": _w_dr_layout(w_qkv[:, 2 * D + h0:2 * D + h0 + HDIM]),
            "wp": np.ascontiguousarray(wp_r.reshape(128, -1)),
            "ident_c": _IDENT,
        })
    return in_maps


def combine_outputs(x, results):
    x = np.asarray(x, np.float32)
    out = np.empty((B, N, D), np.float32)
    for b in range(B):
        acc = x[b].astype(np.float64)
        for g in range(TP):
            acc += results[b * TP + g]["pT"].T / (WSCALE * WSCALE)
        out[b] = acc.astype(np.float32)
    return out


def kernel(x, w_qkv, w_proj):
    nc = _get_nc()
    in_maps = make_in_maps(x, w_qkv, w_proj)
    res = run_bass_kernel_spmd(nc, in_maps, list(range(N_CORES))).results
    return combine_outputs(x, res)


# revision 30
# speedup vs baseline: 1.0112x; 1.0112x over previous
"""Multi-head self-attention (B=2, N=2048, D=1024, H=16) on 8 Trainium2 cores.

Sharding: core c -> batch b = c // 4, head group g = c % 4 (heads 4g..4g+3).
Each core computes q/k/v for its 4 heads (bf16), attention with scores^T in
[j, i] layout, then PV in the "e-as-weights" form: for each 128-wide i-chunk,
matmul(lhsT=e[j, i-chunk], rhs=[v | ones]) accumulates out[i, d] and the
softmax denominator (65th column) in PSUM across the 16 j-chunks.  Normalize
is a per-partition reciprocal broadcast on DVE, transposed back to [d, i] via
an identity matmul into spare PSUM columns, and projected with both head
pairs accumulated in one PSUM group (K=256).  Host sums the 4 per-core
partial projections (+ residual x) per batch.
"""

import numpy as np
import ml_dtypes

import concourse.bass as bass
import concourse.bacc as bacc
import concourse.mybir as mybir
import concourse.tile as tile
from concourse.bass_utils import run_bass_kernel_spmd

B = 2
N = 2048
D = 1024
NH = 16
DH = 64
N_CORES = 8
TP = 4                # head-parallel ways per batch
HPC = NH // TP        # heads per core
HDIM = HPC * DH       # 256 head dims per core
PAIRS = HPC // 2
SCALE = 1.0 / 8.0     # 1/sqrt(DH)

IT = N // 512         # 4 i-tiles
JT = N // 128         # 16 j-chunks
KC = D // 128         # 8 feature chunks

F32 = mybir.dt.float32
BF16 = mybir.dt.bfloat16
F8 = mybir.dt.float8e4
NP_BF16 = ml_dtypes.bfloat16
NP_F8 = ml_dtypes.float8_e4m3
WSCALE = 64.0          # host scales w_qkv/w_proj by this; exp scale and the
                       # host-side combine divide it back out
SCALE_DEV = SCALE / (WSCALE * WSCALE)   # exactly 2**-15
AF = mybir.ActivationFunctionType


def build_bass():
    nc = bacc.Bacc("TRN2", target_bir_lowering=False, debug=False)
    xT = nc.declare_dram_parameter("xT", [D, N], F8, isOutput=False)
    # weights arrive pre-arranged in the on-chip layout: [128, KC/2, 2, HDIM]
    # (DoubleRow pair planes) flattened to 2KB rows for full-speed DMA
    wq = nc.declare_dram_parameter("wq", [128, D * HDIM // 128], F8, isOutput=False)
    wk = nc.declare_dram_parameter("wk", [128, D * HDIM // 128], F8, isOutput=False)
    wv = nc.declare_dram_parameter("wv", [128, D * HDIM // 128], F8, isOutput=False)
    wp = nc.declare_dram_parameter("wp", [128, PAIRS * D], F8, isOutput=False)
    ident_d = nc.declare_dram_parameter("ident_c", [128, 128], BF16, isOutput=False)
    pT = nc.declare_dram_parameter("pT", [D, N], BF16, isOutput=True)

    with tile.TileContext(nc) as tc:
        with (
            tc.tile_pool(name="big", bufs=1) as big,
            tc.tile_pool(name="exps", bufs=6) as exps,
            tc.tile_pool(name="evac", bufs=4) as evac,
            tc.tile_pool(name="psum", bufs=1, space="PSUM") as psum,
        ):
            # ---- input loads: x chunks + wk first (kT p0 is the critical
            # path to the first scores), then wq, then the rest.
            xts = big.tile([128, KC, N], F8, tag="xts")
            xT_r = xT.rearrange("(c p) n -> p c n", p=128)
            wk_sb = big.tile([128, KC // 2, 2, HDIM], F8, tag="wk")
            wq_sb = big.tile([128, KC // 2, 2, HDIM], F8, tag="wq")
            wv_sb = big.tile([128, KC // 2, 2, HDIM], F8, tag="wv")
            wp_sb = big.tile([128, PAIRS, D], F8, tag="wp")
            ident = big.tile([128, 128], BF16, tag="ident")

            nc.sync.dma_start(out=xts[:, 0:2, :], in_=xT_r[:, 0:2, :])
            nc.scalar.dma_start(
                out=wk_sb, in_=wk.rearrange("p (c two w) -> p c two w", two=2, w=HDIM)
            )
            nc.scalar.dma_start(out=xts[:, 2:4, :], in_=xT_r[:, 2:4, :])
            nc.scalar.dma_start(
                out=wq_sb, in_=wq.rearrange("p (c two w) -> p c two w", two=2, w=HDIM)
            )
            nc.sync.dma_start(out=xts[:, 4:6, :], in_=xT_r[:, 4:6, :])
            nc.scalar.dma_start(
                out=wv_sb, in_=wv.rearrange("p (c two w) -> p c two w", two=2, w=HDIM)
            )
            nc.sync.dma_start(out=xts[:, 6:8, :], in_=xT_r[:, 6:8, :])
            nc.sync.dma_start(
                out=wp_sb, in_=wp.rearrange("p (c d) -> p c d", c=PAIRS)
            )
            nc.sync.dma_start(out=ident, in_=ident_d[:, :])

            # v_aug[j, jt, head, 0:64] = v, [..., 64] = 1.0 (denominator trick)
            v_aug = big.tile([128, JT, HPC, 65], BF16, tag="v")
            nc.vector.memset(v_aug[:, :, :, 64:65], 1.0)

            e_set0 = big.tile([128, JT, 1024], BF16, tag="eset0")
            e_set1 = big.tile([128, JT, 1024], BF16, tag="eset1")
            e_sets = [e_set0, e_set1]
            qT = big.tile([128, PAIRS, N], BF16, tag="qT")
            kT = big.tile([128, PAIRS, N], BF16, tag="kT")
            outT = big.tile([128, PAIRS, N], F8, tag="outT")

            warm = evac.tile([1, 1], F32, tag="warm")
            nc.scalar.activation(warm, ident[0:1, 0:1], AF.Exp)

            def emit_qk_tile(p, w_sb, dst, nt):
                ps = psum.tile([128, 512], F32, tag="mm", bufs=2)
                for c in range(KC // 2):
                    nc.tensor.matmul(
                        ps,
                        lhsT=w_sb[:, c, :, p * 128:(p + 1) * 128],
                        rhs=xts[:, 2 * c:2 * c + 2, nt * 512:(nt + 1) * 512],
                        start=(c == 0),
                        stop=(c == KC // 2 - 1),
                        perf_mode=mybir.MatmulPerfMode.DoubleRow,
                    )
                nc.vector.tensor_copy(dst[:, p, nt * 512:(nt + 1) * 512], ps)

            def emit_v_chunk(t):
                ps = psum.tile([128, 512], F32, tag="mm", bufs=2)
                for c in range(KC // 2):
                    nc.tensor.matmul(
                        ps[:, 0:HDIM],
                        lhsT=xts[:, 2 * c:2 * c + 2, t * 128:(t + 1) * 128],
                        rhs=wv_sb[:, c, :, :],
                        start=(c == 0),
                        stop=(c == KC // 2 - 1),
                        perf_mode=mybir.MatmulPerfMode.DoubleRow,
                    )
                nc.vector.tensor_copy(
                    v_aug[:, t, :, 0:64],
                    ps[:, 0:HDIM].rearrange("p (h c) -> p h c", c=64),
                )

            def emit_scores(p, it, jt):
                sc = psum.tile([128, 1024], F32, tag="sc", bufs=2)
                for h in range(2):
                    nc.tensor.matmul(
                        sc[:, h * 512:(h + 1) * 512],
                        lhsT=kT[h * 64:(h + 1) * 64, p, jt * 128:(jt + 1) * 128],
                        rhs=qT[h * 64:(h + 1) * 64, p, it * 512:(it + 1) * 512],
                        start=True,
                        stop=True,
                    )
                return sc

            def emit_pv_group(p, g, eset, pvt):
                # one accumulation group (h, ic): 16 back-to-back matmuls over
                # the j-chunks -- exactly one open group per PSUM bank.
                h, ic = divmod(g, 4)
                for jt in range(JT):
                    nc.tensor.matmul(
                        pvt[:, h * 512 + ic * 65:h * 512 + ic * 65 + 65],
                        lhsT=eset[:, jt, h * 512 + ic * 128:h * 512 + (ic + 1) * 128],
                        rhs=v_aug[:, jt, 2 * p + h, :],
                        start=(jt == 0),
                        stop=(jt == JT - 1),
                    )

            def emit_norm_transpose(p, it, pvt):
                # All pvt readers (recip + muls) come first so the next
                # window's PV (WAR on the single pv buffer) unblocks early;
                # transposes land in mm-pool scratch, then evac to outT.
                rc = evac.tile([128, 8], F32, tag="rc", bufs=2)
                dg = evac.tile([128, 8], F32, tag="dg", bufs=2)
                nc.vector.tensor_copy(
                    dg[:, 0:4],
                    pvt[:, 0:260].rearrange("p (g c) -> p g c", c=65)[:, :, 64],
                )
                nc.vector.tensor_copy(
                    dg[:, 4:8],
                    pvt[:, 512:772].rearrange("p (g c) -> p g c", c=65)[:, :, 64],
                )
                nc.vector.reciprocal(rc, dg)
                ois = []
                for ic in range(4):
                    oi = evac.tile([128, 128], BF16, tag="outI", bufs=4)
                    nc.vector.tensor_scalar_mul(
                        oi[:, 0:64],
                        pvt[:, ic * 65:ic * 65 + 64],
                        rc[:, ic:ic + 1],
                    )
                    nc.vector.tensor_scalar_mul(
                        oi[:, 64:128],
                        pvt[:, 512 + ic * 65:512 + ic * 65 + 64],
                        rc[:, 4 + ic:4 + ic + 1],
                    )
                    ois.append(oi)
                ps = psum.tile([128, 512], F32, tag="mm", bufs=2)
                for ic in range(4):
                    nc.tensor.matmul(
                        ps[:, ic * 128:(ic + 1) * 128],
                        lhsT=ois[ic], rhs=ident, start=True, stop=True,
                    )
                for ic in range(4):
                    nc.vector.tensor_copy(
                        outT[:, p, it * 512 + ic * 128:it * 512 + (ic + 1) * 128],
                        ps[:, ic * 128:(ic + 1) * 128],
                    )

            def emit_proj_tile(it, ot, slot=None):
                if slot is not None:
                    pj = slot
                else:
                    pj = psum.tile([128, 512], F32, tag="mm", bufs=2)
                nc.tensor.matmul(
                    pj,
                    lhsT=wp_sb[:, :, ot * 128:(ot + 1) * 128],
                    rhs=outT[:, :, it * 512:(it + 1) * 512],
                    start=True,
                    stop=True,
                    perf_mode=mybir.MatmulPerfMode.DoubleRow,
                )
                o_sb = evac.tile([128, 512], BF16, tag="osb", bufs=8)
                if slot is not None and ot % 2 == 1:
                    nc.scalar.copy(o_sb, pj)
                else:
                    nc.vector.tensor_copy(o_sb, pj)
                deng = nc.scalar if (slot is not None and ot % 2 == 0) else nc.sync
                deng.dma_start(
                    out=pT[ot * 128:(ot + 1) * 128, it * 512:(it + 1) * 512],
                    in_=o_sb,
                )

            # ---- prelude: kT p0 chunk for jt 0-3, qT p0 it0 ----
            emit_qk_tile(0, wk_sb, kT, 0)
            emit_qk_tile(0, wq_sb, qT, 0)

            # filler work queue: (kind, pair, arg), ordered by first use.
            fillers = [
                ("k", 0, 1), ("q", 0, 1), ("k", 0, 2), ("k", 0, 3),
                ("k", 1, 0), ("q", 0, 2), ("k", 1, 1), ("q", 1, 0),
                ("k", 1, 2), ("k", 1, 3), ("q", 0, 3), ("q", 1, 1),
                ("q", 1, 2), ("q", 1, 3),
            ]
            proj_q = []

            def pop_filler():
                if fillers:
                    kind, p_, arg = fillers.pop(0)
                    emit_qk_tile(p_, wk_sb if kind == "k" else wq_sb,
                                 kT if kind == "k" else qT, arg)
                    return True
                if proj_q:
                    it_t, ot = proj_q.pop(0)
                    emit_proj_tile(it_t, ot)
                    return True
                return False

            # Flattened step stream over all (pair, i-tile, j-chunk) steps.
            # Scores are emitted one step ahead (across window boundaries).
            # exp(w, jt) fills e_sets[w % 2]; the PV of window w-1 runs as 8
            # sequential accumulation groups (one per PSUM bank at a time)
            # spread over the first steps of window w, followed by its
            # normalize/transpose chain and projection availability.
            windows = [(p, it) for p in range(PAIRS) for it in range(IT)]
            NW = len(windows)
            steps = NW * JT

            def step_scores(s):
                w, jt = divmod(s, JT)
                p, it = windows[w]
                return emit_scores(p, it, jt)

            prev = None          # (p, it, pvt, eset) of the previous window
            pvt = None
            sc_cur = step_scores(0)
            for s in range(steps):
                w, jt = divmod(s, JT)
                p, it = windows[w]
                eset = e_sets[w % 2]
                if jt == 0:
                    pvt = psum.tile([128, 1024], F32, tag="pv", bufs=1)
                sc = sc_cur
                sc_cur = step_scores(s + 1) if s + 1 < steps else None
                # fillers / v chunks
                if w == 0:
                    emit_v_chunk(jt)
                    if jt in (2, 4, 6, 10):
                        pop_filler()
                elif jt % 2 == 1:
                    pop_filler()
                    if proj_q and jt % 4 == 1:
                        pop_filler()
                nc.scalar.activation(eset[:, jt, :], sc, AF.Exp, scale=SCALE_DEV)
                if prev is not None:
                    if jt < 8:
                        emit_pv_group(prev[0], jt, prev[3], prev[2])
                    elif jt == 8:
                        p0, it0, pvt0, _ = prev
                        emit_norm_transpose(p0, it0, pvt0)
                        if p0 == 1:
                            proj_q.extend((it0, ot) for ot in range(D // 128))
                if jt == JT - 1:
                    prev = (p, it, pvt, eset)

            # ---- drain: last window's remaining PV groups + per-ic
            # normalize pipeline + projection (ACT helps with evacs) ----
            p0, it0, pvt0, eset0 = prev
            while fillers:
                pop_filler()
            # leftover mid-stream proj tiles (outT for them is ready)
            for it_t, ot in proj_q:
                emit_proj_tile(it_t, ot)
            proj_q = []
            tmm = psum.tile([128, 512], F32, tag="mm", bufs=2)

            def norm_ic(ic):
                rc = evac.tile([128, 2], F32, tag="rcd", bufs=4)
                nc.vector.reciprocal(
                    rc[:, 0:1], pvt0[:, ic * 65 + 64:ic * 65 + 65]
                )
                nc.vector.reciprocal(
                    rc[:, 1:2], pvt0[:, 512 + ic * 65 + 64:512 + ic * 65 + 65]
                )
                oi = evac.tile([128, 128], BF16, tag="outI", bufs=4)
                nc.vector.tensor_scalar_mul(
                    oi[:, 0:64], pvt0[:, ic * 65:ic * 65 + 64], rc[:, 0:1]
                )
                nc.vector.tensor_scalar_mul(
                    oi[:, 64:128],
                    pvt0[:, 512 + ic * 65:512 + ic * 65 + 64], rc[:, 1:2]
                )
                nc.tensor.matmul(
                    tmm[:, ic * 128:(ic + 1) * 128],
                    lhsT=oi, rhs=ident, start=True, stop=True,
                )
                nc.vector.tensor_copy(
                    outT[:, p0, it0 * 512 + ic * 128:it0 * 512 + (ic + 1) * 128],
                    tmm[:, ic * 128:(ic + 1) * 128],
                )

            # interleave: group burst ic+1 || norm chain ic || proj slice
            # (ot, ic-1).  proj is done per 128-i chunk so each norm wave
            # immediately feeds its projection; evacs alternate DVE/ACT and
            # each ot's [128, 512] staging row is DMA'd once complete.
            sct0 = psum.tile([128, 1024], F32, tag="sc", bufs=2)
            sct1 = psum.tile([128, 1024], F32, tag="sc", bufs=2)
            pj_slot = [sct0[:, 0:512], sct0[:, 512:1024],
                       sct1[:, 0:512], sct1[:, 512:1024],
                       tmm[:, 0:512]][:4]
            osbs = []
            for _oi in range(8):
                osb_d = evac.tile([128, 512], BF16, tag="osb", bufs=8, name=f"osbd{_oi}")
                osbs.append(osb_d)

            emit_pv_group(p0, 0, eset0, pvt0)
            emit_pv_group(p0, 4, eset0, pvt0)
            for ic in range(4):
                if ic + 1 < 4:
                    emit_pv_group(p0, ic + 1, eset0, pvt0)
                    emit_pv_group(p0, 4 + ic + 1, eset0, pvt0)
                norm_ic(ic)
                for ot in range(8):
                    slot = pj_slot[ot % 4]
                    out_ps = slot[:, (ot // 4) * 256 + (ic % 2) * 128:
                                  (ot // 4) * 256 + (ic % 2) * 128 + 128]
                    nc.tensor.matmul(
                        out_ps,
                        lhsT=wp_sb[:, :, ot * 128:(ot + 1) * 128],
                        rhs=outT[:, :, it0 * 512 + ic * 128:
                                 it0 * 512 + (ic + 1) * 128],
                        start=True, stop=True,
                        perf_mode=mybir.MatmulPerfMode.DoubleRow,
                    )
                    ceng = nc.scalar if ot % 2 else nc.vector
                    ceng_copy = ceng.copy if ot % 2 else ceng.tensor_copy
                    ceng_copy(osbs[ot][:, ic * 128:(ic + 1) * 128], out_ps)
                    if ic == 3:
                        deng = nc.scalar if ot % 2 else nc.sync
                        deng.dma_start(
                            out=pT[ot * 128:(ot + 1) * 128,
                                   it0 * 512:(it0 + 1) * 512],
                            in_=osbs[ot],
                        )
            proj_q = []
    return nc


_NC = None


def _get_nc():
    global _NC
    if _NC is None:
        _NC = build_bass()
        _NC.finalize()
    return _NC


_IDENT = np.eye(128, dtype=NP_BF16)


def _w_dr_layout(w):
    # [D, HDIM] -> [128, KC/2 * 2 * HDIM]: row p, flat (c, plane, m) with
    # plane = DoubleRow pair index; element = w[(2c + plane)*128 + p, m]
    r = w.reshape(KC // 2, 2, 128, HDIM).transpose(2, 0, 1, 3)
    return np.ascontiguousarray(r.reshape(128, -1))


def make_in_maps(x, w_qkv, w_proj):
    x = np.asarray(x, np.float32)
    w_qkv = (np.asarray(w_qkv, np.float32) * WSCALE).astype(NP_F8)
    w_proj = (np.asarray(w_proj, np.float32) * WSCALE).astype(NP_F8)
    xTs = [np.ascontiguousarray(x[b].T.astype(NP_F8)) for b in range(B)]
    in_maps = []
    for c in range(N_CORES):
        b, g = divmod(c, TP)
        h0 = g * HDIM
        wp_r = w_proj[h0:h0 + HDIM, :].reshape(PAIRS, 128, D).transpose(1, 0, 2)
        in_maps.append({
            "xT": xTs[b],
            "wq": _w_dr_layout(w_qkv[:, h0:h0 + HDIM]),
            "wk": _w_dr_layout(w_qkv[:, D + h0:D + h0 + HDIM]),
            "wv": _w_dr_layout(w_qkv[:, 2 * D + h0:2 * D + h0 + HDIM]),
            "wp": np.ascontiguousarray(wp_r.reshape(128, -1)),
            "ident_c": _IDENT,
        })
    return in_maps


def combine_outputs(x, results):
    x = np.asarray(x, np.float32)
    out = np.empty((B, N, D), np.float32)
    for b in range(B):
        acc = x[b].astype(np.float64)
        for g in range(TP):
            acc += results[b * TP + g]["pT"].astype(np.float32).T / (WSCALE * WSCALE)
        out[b] = acc.astype(np.float32)
    return out


def kernel(x, w_qkv, w_proj):
    nc = _get_nc()
    in_maps = make_in_maps(x, w_qkv, w_proj)
    res = run_bass_kernel_spmd(nc, in_maps, list(range(N_CORES))).results
    return combine_outputs(x, res)


# revision 31
# speedup vs baseline: 1.0241x; 1.0128x over previous
"""Multi-head self-attention (B=2, N=2048, D=1024, H=16) on 8 Trainium2 cores.

Sharding: core c -> batch b = c // 4, head group g = c % 4 (heads 4g..4g+3).
Each core computes q/k/v for its 4 heads (bf16), attention with scores^T in
[j, i] layout, then PV in the "e-as-weights" form: for each 128-wide i-chunk,
matmul(lhsT=e[j, i-chunk], rhs=[v | ones]) accumulates out[i, d] and the
softmax denominator (65th column) in PSUM across the 16 j-chunks.  Normalize
is a per-partition reciprocal broadcast on DVE, transposed back to [d, i] via
an identity matmul into spare PSUM columns, and projected with both head
pairs accumulated in one PSUM group (K=256).  Host sums the 4 per-core
partial projections (+ residual x) per batch.
"""

import numpy as np
import ml_dtypes

import concourse.bass as bass
import concourse.bacc as bacc
import concourse.mybir as mybir
import concourse.tile as tile
from concourse.bass_utils import run_bass_kernel_spmd

B = 2
N = 2048
D = 1024
NH = 16
DH = 64
N_CORES = 8
TP = 4                # head-parallel ways per batch
HPC = NH // TP        # heads per core
HDIM = HPC * DH       # 256 head dims per core
PAIRS = HPC // 2
SCALE = 1.0 / 8.0     # 1/sqrt(DH)

IT = N // 512         # 4 i-tiles
JT = N // 128         # 16 j-chunks
KC = D // 128         # 8 feature chunks

F32 = mybir.dt.float32
BF16 = mybir.dt.bfloat16
F8 = mybir.dt.float8e4
NP_BF16 = ml_dtypes.bfloat16
NP_F8 = ml_dtypes.float8_e4m3
WSCALE = 64.0          # host scales w_qkv/w_proj by this; exp scale and the
                       # host-side combine divide it back out
SCALE_DEV = SCALE / (WSCALE * WSCALE)   # exactly 2**-15
AF = mybir.ActivationFunctionType


def build_bass():
    nc = bacc.Bacc("TRN2", target_bir_lowering=False, debug=False)
    xT = nc.declare_dram_parameter("xT", [D, N], F8, isOutput=False)
    # weights arrive pre-arranged in the on-chip layout: [128, KC/2, 2, HDIM]
    # (DoubleRow pair planes) flattened to 2KB rows for full-speed DMA
    wq = nc.declare_dram_parameter("wq", [128, D * HDIM // 128], F8, isOutput=False)
    wk = nc.declare_dram_parameter("wk", [128, D * HDIM // 128], F8, isOutput=False)
    wv = nc.declare_dram_parameter("wv", [128, D * HDIM // 128], F8, isOutput=False)
    wp = nc.declare_dram_parameter("wp", [128, PAIRS * D], F8, isOutput=False)
    ident_d = nc.declare_dram_parameter("ident_c", [128, 128], BF16, isOutput=False)
    pT = nc.declare_dram_parameter("pT", [D, N], BF16, isOutput=True)

    with tile.TileContext(nc) as tc:
        with (
            tc.tile_pool(name="big", bufs=1) as big,
            tc.tile_pool(name="exps", bufs=6) as exps,
            tc.tile_pool(name="evac", bufs=4) as evac,
            tc.tile_pool(name="psum", bufs=1, space="PSUM") as psum,
        ):
            # ---- input loads: x chunks + wk first (kT p0 is the critical
            # path to the first scores), then wq, then the rest.
            xts = big.tile([128, KC, N], F8, tag="xts")
            xT_r = xT.rearrange("(c p) n -> p c n", p=128)
            wk_sb = big.tile([128, KC // 2, 2, HDIM], F8, tag="wk")
            wq_sb = big.tile([128, KC // 2, 2, HDIM], F8, tag="wq")
            wv_sb = big.tile([128, KC // 2, 2, HDIM], F8, tag="wv")
            wp_sb = big.tile([128, PAIRS, D], F8, tag="wp")
            ident = big.tile([128, 128], BF16, tag="ident")

            # x arrives in TOKEN slices: the first 512 tokens cover both
            # kT(jt 0-3) and qT(it0) for the whole feature contraction, so
            # the first exp fires after ~2 DMAs instead of the full 2MB of x.
            nc.scalar.dma_start(
                out=wk_sb, in_=wk.rearrange("p (c two w) -> p c two w", two=2, w=HDIM)
            )
            nc.sync.dma_start(out=xts[:, :, 0:512], in_=xT_r[:, :, 0:512])
            nc.scalar.dma_start(
                out=wq_sb, in_=wq.rearrange("p (c two w) -> p c two w", two=2, w=HDIM)
            )
            nc.sync.dma_start(out=xts[:, :, 512:1024], in_=xT_r[:, :, 512:1024])
            nc.scalar.dma_start(
                out=wv_sb, in_=wv.rearrange("p (c two w) -> p c two w", two=2, w=HDIM)
            )
            nc.sync.dma_start(out=xts[:, :, 1024:1536], in_=xT_r[:, :, 1024:1536])
            nc.sync.dma_start(out=xts[:, :, 1536:2048], in_=xT_r[:, :, 1536:2048])
            nc.scalar.dma_start(
                out=wp_sb, in_=wp.rearrange("p (c d) -> p c d", c=PAIRS)
            )
            nc.sync.dma_start(out=ident, in_=ident_d[:, :])

            # v_aug[j, jt, head, 0:64] = v, [..., 64] = 1.0 (denominator trick)
            v_aug = big.tile([128, JT, HPC, 65], BF16, tag="v")
            nc.vector.memset(v_aug[:, :, :, 64:65], 1.0)

            e_set0 = big.tile([128, JT, 1024], BF16, tag="eset0")
            e_set1 = big.tile([128, JT, 1024], BF16, tag="eset1")
            e_sets = [e_set0, e_set1]
            qT = big.tile([128, PAIRS, N], BF16, tag="qT")
            kT = big.tile([128, PAIRS, N], BF16, tag="kT")
            outT = big.tile([128, PAIRS, N], F8, tag="outT")

            warm = evac.tile([1, 1], F32, tag="warm")
            nc.scalar.activation(warm, ident[0:1, 0:1], AF.Exp)

            def emit_qk_tile(p, w_sb, dst, nt):
                ps = psum.tile([128, 512], F32, tag="mm", bufs=2)
                for c in range(KC // 2):
                    nc.tensor.matmul(
                        ps,
                        lhsT=w_sb[:, c, :, p * 128:(p + 1) * 128],
                        rhs=xts[:, 2 * c:2 * c + 2, nt * 512:(nt + 1) * 512],
                        start=(c == 0),
                        stop=(c == KC // 2 - 1),
                        perf_mode=mybir.MatmulPerfMode.DoubleRow,
                    )
                nc.vector.tensor_copy(dst[:, p, nt * 512:(nt + 1) * 512], ps)

            def emit_v_chunk(t):
                ps = psum.tile([128, 512], F32, tag="mm", bufs=2)
                for c in range(KC // 2):
                    nc.tensor.matmul(
                        ps[:, 0:HDIM],
                        lhsT=xts[:, 2 * c:2 * c + 2, t * 128:(t + 1) * 128],
                        rhs=wv_sb[:, c, :, :],
                        start=(c == 0),
                        stop=(c == KC // 2 - 1),
                        perf_mode=mybir.MatmulPerfMode.DoubleRow,
                    )
                nc.vector.tensor_copy(
                    v_aug[:, t, :, 0:64],
                    ps[:, 0:HDIM].rearrange("p (h c) -> p h c", c=64),
                )

            def emit_scores(p, it, jt):
                sc = psum.tile([128, 1024], F32, tag="sc", bufs=2)
                for h in range(2):
                    nc.tensor.matmul(
                        sc[:, h * 512:(h + 1) * 512],
                        lhsT=kT[h * 64:(h + 1) * 64, p, jt * 128:(jt + 1) * 128],
                        rhs=qT[h * 64:(h + 1) * 64, p, it * 512:(it + 1) * 512],
                        start=True,
                        stop=True,
                    )
                return sc

            def emit_pv_group(p, g, eset, pvt):
                # one accumulation group (h, ic): 16 back-to-back matmuls over
                # the j-chunks -- exactly one open group per PSUM bank.
                h, ic = divmod(g, 4)
                for jt in range(JT):
                    nc.tensor.matmul(
                        pvt[:, h * 512 + ic * 65:h * 512 + ic * 65 + 65],
                        lhsT=eset[:, jt, h * 512 + ic * 128:h * 512 + (ic + 1) * 128],
                        rhs=v_aug[:, jt, 2 * p + h, :],
                        start=(jt == 0),
                        stop=(jt == JT - 1),
                    )

            def emit_norm_transpose(p, it, pvt):
                # All pvt readers (recip + muls) come first so the next
                # window's PV (WAR on the single pv buffer) unblocks early;
                # transposes land in mm-pool scratch, then evac to outT.
                rc = evac.tile([128, 8], F32, tag="rc", bufs=2)
                dg = evac.tile([128, 8], F32, tag="dg", bufs=2)
                nc.vector.tensor_copy(
                    dg[:, 0:4],
                    pvt[:, 0:260].rearrange("p (g c) -> p g c", c=65)[:, :, 64],
                )
                nc.vector.tensor_copy(
                    dg[:, 4:8],
                    pvt[:, 512:772].rearrange("p (g c) -> p g c", c=65)[:, :, 64],
                )
                nc.vector.reciprocal(rc, dg)
                ois = []
                for ic in range(4):
                    oi = evac.tile([128, 128], BF16, tag="outI", bufs=4)
                    nc.vector.tensor_scalar_mul(
                        oi[:, 0:64],
                        pvt[:, ic * 65:ic * 65 + 64],
                        rc[:, ic:ic + 1],
                    )
                    nc.vector.tensor_scalar_mul(
                        oi[:, 64:128],
                        pvt[:, 512 + ic * 65:512 + ic * 65 + 64],
                        rc[:, 4 + ic:4 + ic + 1],
                    )
                    ois.append(oi)
                ps = psum.tile([128, 512], F32, tag="mm", bufs=2)
                for ic in range(4):
                    nc.tensor.matmul(
                        ps[:, ic * 128:(ic + 1) * 128],
                        lhsT=ois[ic], rhs=ident, start=True, stop=True,
                    )
                for ic in range(4):
                    nc.vector.tensor_copy(
                        outT[:, p, it * 512 + ic * 128:it * 512 + (ic + 1) * 128],
                        ps[:, ic * 128:(ic + 1) * 128],
                    )

            def emit_proj_tile(it, ot, slot=None):
                if slot is not None:
                    pj = slot
                else:
                    pj = psum.tile([128, 512], F32, tag="mm", bufs=2)
                nc.tensor.matmul(
                    pj,
                    lhsT=wp_sb[:, :, ot * 128:(ot + 1) * 128],
                    rhs=outT[:, :, it * 512:(it + 1) * 512],
                    start=True,
                    stop=True,
                    perf_mode=mybir.MatmulPerfMode.DoubleRow,
                )
                o_sb = evac.tile([128, 512], BF16, tag="osb", bufs=8)
                if slot is not None and ot % 2 == 1:
                    nc.scalar.copy(o_sb, pj)
                else:
                    nc.vector.tensor_copy(o_sb, pj)
                deng = nc.scalar if (slot is not None and ot % 2 == 0) else nc.sync
                deng.dma_start(
                    out=pT[ot * 128:(ot + 1) * 128, it * 512:(it + 1) * 512],
                    in_=o_sb,
                )

            # ---- prelude: kT p0 chunk for jt 0-3, qT p0 it0 ----
            emit_qk_tile(0, wk_sb, kT, 0)
            emit_qk_tile(0, wq_sb, qT, 0)

            # filler work queue: (kind, pair, arg), ordered by first use.
            fillers = [
                ("k", 0, 1), ("q", 0, 1), ("k", 0, 2), ("k", 0, 3),
                ("k", 1, 0), ("q", 0, 2), ("k", 1, 1), ("q", 1, 0),
                ("k", 1, 2), ("k", 1, 3), ("q", 0, 3), ("q", 1, 1),
                ("q", 1, 2), ("q", 1, 3),
            ]
            proj_q = []

            def pop_filler():
                if fillers:
                    kind, p_, arg = fillers.pop(0)
                    emit_qk_tile(p_, wk_sb if kind == "k" else wq_sb,
                                 kT if kind == "k" else qT, arg)
                    return True
                if proj_q:
                    it_t, ot = proj_q.pop(0)
                    emit_proj_tile(it_t, ot)
                    return True
                return False

            # Flattened step stream over all (pair, i-tile, j-chunk) steps.
            # Scores are emitted one step ahead (across window boundaries).
            # exp(w, jt) fills e_sets[w % 2]; the PV of window w-1 runs as 8
            # sequential accumulation groups (one per PSUM bank at a time)
            # spread over the first steps of window w, followed by its
            # normalize/transpose chain and projection availability.
            windows = [(p, it) for p in range(PAIRS) for it in range(IT)]
            NW = len(windows)
            steps = NW * JT

            def step_scores(s):
                w, jt = divmod(s, JT)
                p, it = windows[w]
                return emit_scores(p, it, jt)

            prev = None          # (p, it, pvt, eset) of the previous window
            pvt = None
            sc_cur = step_scores(0)
            for s in range(steps):
                w, jt = divmod(s, JT)
                p, it = windows[w]
                eset = e_sets[w % 2]
                if jt == 0:
                    pvt = psum.tile([128, 1024], F32, tag="pv", bufs=1)
                sc = sc_cur
                sc_cur = step_scores(s + 1) if s + 1 < steps else None
                # fillers / v chunks
                if w == 0:
                    emit_v_chunk(jt)
                    if jt in (2, 4, 6, 10):
                        pop_filler()
                elif jt % 2 == 1:
                    pop_filler()
                    if proj_q and jt % 4 == 1:
                        pop_filler()
                nc.scalar.activation(eset[:, jt, :], sc, AF.Exp, scale=SCALE_DEV)
                if prev is not None:
                    if jt < 8:
                        emit_pv_group(prev[0], jt, prev[3], prev[2])
                    elif jt == 8:
                        p0, it0, pvt0, _ = prev
                        emit_norm_transpose(p0, it0, pvt0)
                        if p0 == 1:
                            proj_q.extend((it0, ot) for ot in range(D // 128))
                if jt == JT - 1:
                    prev = (p, it, pvt, eset)

            # ---- drain: last window's remaining PV groups + per-ic
            # normalize pipeline + projection (ACT helps with evacs) ----
            p0, it0, pvt0, eset0 = prev
            while fillers:
                pop_filler()
            # leftover mid-stream proj tiles (outT for them is ready)
            for it_t, ot in proj_q:
                emit_proj_tile(it_t, ot)
            proj_q = []
            tmm = psum.tile([128, 512], F32, tag="mm", bufs=2)

            def norm_ic(ic):
                rc = evac.tile([128, 2], F32, tag="rcd", bufs=4)
                nc.vector.reciprocal(
                    rc[:, 0:1], pvt0[:, ic * 65 + 64:ic * 65 + 65]
                )
                nc.vector.reciprocal(
                    rc[:, 1:2], pvt0[:, 512 + ic * 65 + 64:512 + ic * 65 + 65]
                )
                oi = evac.tile([128, 128], BF16, tag="outI", bufs=4)
                nc.vector.tensor_scalar_mul(
                    oi[:, 0:64], pvt0[:, ic * 65:ic * 65 + 64], rc[:, 0:1]
                )
                nc.vector.tensor_scalar_mul(
                    oi[:, 64:128],
                    pvt0[:, 512 + ic * 65:512 + ic * 65 + 64], rc[:, 1:2]
                )
                nc.tensor.matmul(
                    tmm[:, ic * 128:(ic + 1) * 128],
                    lhsT=oi, rhs=ident, start=True, stop=True,
                )
                nc.vector.tensor_copy(
                    outT[:, p0, it0 * 512 + ic * 128:it0 * 512 + (ic + 1) * 128],
                    tmm[:, ic * 128:(ic + 1) * 128],
                )

            # interleave: group burst ic+1 || norm chain ic || proj slice
            # (ot, ic-1).  proj is done per 128-i chunk so each norm wave
            # immediately feeds its projection; evacs alternate DVE/ACT and
            # each ot's [128, 512] staging row is DMA'd once complete.
            sct0 = psum.tile([128, 1024], F32, tag="sc", bufs=2)
            sct1 = psum.tile([128, 1024], F32, tag="sc", bufs=2)
            pj_slot = [sct0[:, 0:512], sct0[:, 512:1024],
                       sct1[:, 0:512], sct1[:, 512:1024],
                       tmm[:, 0:512]][:4]
            osbs = []
            for _oi in range(8):
                osb_d = evac.tile([128, 512], BF16, tag="osb", bufs=8, name=f"osbd{_oi}")
                osbs.append(osb_d)

            emit_pv_group(p0, 0, eset0, pvt0)
            emit_pv_group(p0, 4, eset0, pvt0)
            for ic in range(4):
                if ic + 1 < 4:
                    emit_pv_group(p0, ic + 1, eset0, pvt0)
                    emit_pv_group(p0, 4 + ic + 1, eset0, pvt0)
                norm_ic(ic)
                for ot in range(8):
                    slot = pj_slot[ot % 4]
                    out_ps = slot[:, (ot // 4) * 256 + (ic % 2) * 128:
                                  (ot // 4) * 256 + (ic % 2) * 128 + 128]
                    nc.tensor.matmul(
                        out_ps,
                        lhsT=wp_sb[:, :, ot * 128:(ot + 1) * 128],
                        rhs=outT[:, :, it0 * 512 + ic * 128:
                                 it0 * 512 + (ic + 1) * 128],
                        start=True, stop=True,
                        perf_mode=mybir.MatmulPerfMode.DoubleRow,
                    )
                    ceng = nc.scalar if ot % 2 else nc.vector
                    ceng_copy = ceng.copy if ot % 2 else ceng.tensor_copy
                    ceng_copy(osbs[ot][:, ic * 128:(ic + 1) * 128], out_ps)
                    if ic == 3:
                        deng = nc.scalar if ot % 2 else nc.sync
                        deng.dma_start(
                            out=pT[ot * 128:(ot + 1) * 128,
                                   it0 * 512:(it0 + 1) * 512],
                            in_=osbs[ot],
                        )
            proj_q = []
    return nc


_NC = None


def _get_nc():
    global _NC
    if _NC is None:
        _NC = build_bass()
        _NC.finalize()
    return _NC


_IDENT = np.eye(128, dtype=NP_BF16)


def _w_dr_layout(w):
    # [D, HDIM] -> [128, KC/2 * 2 * HDIM]: row p, flat (c, plane, m) with
    # plane = DoubleRow pair index; element = w[(2c + plane)*128 + p, m]
    r = w.reshape(KC // 2, 2, 128, HDIM).transpose(2, 0, 1, 3)
    return np.ascontiguousarray(r.reshape(128, -1))


def make_in_maps(x, w_qkv, w_proj):
    x = np.asarray(x, np.float32)
    w_qkv = (np.asarray(w_qkv, np.float32) * WSCALE).astype(NP_F8)
    w_proj = (np.asarray(w_proj, np.float32) * WSCALE).astype(NP_F8)
    xTs = [np.ascontiguousarray(x[b].T.astype(NP_F8)) for b in range(B)]
    in_maps = []
    for c in range(N_CORES):
        b, g = divmod(c, TP)
        h0 = g * HDIM
        wp_r = w_proj[h0:h0 + HDIM, :].reshape(PAIRS, 128, D).transpose(1, 0, 2)
        in_maps.append({
            "xT": xTs[b],
            "wq": _w_dr_layout(w_qkv[:, h0:h0 + HDIM]),
            "wk": _w_dr_layout(w_qkv[:, D + h0:D + h0 + HDIM]),
            "wv": _w_dr_layout(w_qkv[:, 2 * D + h0:2 * D + h0 + HDIM]),
            "wp": np.ascontiguousarray(wp_r.reshape(128, -1)),
            "ident_c": _IDENT,
        })
    return in_maps


def combine_outputs(x, results):
    x = np.asarray(x, np.float32)
    out = np.empty((B, N, D), np.float32)
    for b in range(B):
        acc = x[b].astype(np.float64)
        for g in range(TP):
            acc += results[b * TP + g]["pT"].astype(np.float32).T / (WSCALE * WSCALE)
        out[b] = acc.astype(np.float32)
    return out


def kernel(x, w_qkv, w_proj):
    nc = _get_nc()
    in_maps = make_in_maps(x, w_qkv, w_proj)
    res = run_bass_kernel_spmd(nc, in_maps, list(range(N_CORES))).results
    return combine_outputs(x, res)


# revision 32
# speedup vs baseline: 1.0309x; 1.0066x over previous
"""Multi-head self-attention (B=2, N=2048, D=1024, H=16) on 8 Trainium2 cores.

Sharding: core c -> batch b = c // 4, head group g = c % 4 (heads 4g..4g+3).
Each core computes q/k/v for its 4 heads (bf16), attention with scores^T in
[j, i] layout, then PV in the "e-as-weights" form: for each 128-wide i-chunk,
matmul(lhsT=e[j, i-chunk], rhs=[v | ones]) accumulates out[i, d] and the
softmax denominator (65th column) in PSUM across the 16 j-chunks.  Normalize
is a per-partition reciprocal broadcast on DVE, transposed back to [d, i] via
an identity matmul into spare PSUM columns, and projected with both head
pairs accumulated in one PSUM group (K=256).  Host sums the 4 per-core
partial projections (+ residual x) per batch.
"""

import numpy as np
import ml_dtypes

import concourse.bass as bass
import concourse.bacc as bacc
import concourse.mybir as mybir
import concourse.tile as tile
from concourse.bass_utils import run_bass_kernel_spmd

B = 2
N = 2048
D = 1024
NH = 16
DH = 64
N_CORES = 8
TP = 4                # head-parallel ways per batch
HPC = NH // TP        # heads per core
HDIM = HPC * DH       # 256 head dims per core
PAIRS = HPC // 2
SCALE = 1.0 / 8.0     # 1/sqrt(DH)

IT = N // 512         # 4 i-tiles
JT = N // 128         # 16 j-chunks
KC = D // 128         # 8 feature chunks

F32 = mybir.dt.float32
BF16 = mybir.dt.bfloat16
F8 = mybir.dt.float8e4
NP_BF16 = ml_dtypes.bfloat16
NP_F8 = ml_dtypes.float8_e4m3
WSCALE = 64.0          # host scales w_qkv/w_proj by this; exp scale and the
                       # host-side combine divide it back out
SCALE_DEV = SCALE / (WSCALE * WSCALE)   # exactly 2**-15
AF = mybir.ActivationFunctionType


def build_bass():
    nc = bacc.Bacc("TRN2", target_bir_lowering=False, debug=False)
    xT = nc.declare_dram_parameter("xT", [D, N], F8, isOutput=False)
    # weights arrive pre-arranged in the on-chip layout: [128, KC/2, 2, HDIM]
    # (DoubleRow pair planes) flattened to 2KB rows for full-speed DMA
    wq = nc.declare_dram_parameter("wq", [128, D * HDIM // 128], F8, isOutput=False)
    wk = nc.declare_dram_parameter("wk", [128, D * HDIM // 128], F8, isOutput=False)
    wv = nc.declare_dram_parameter("wv", [128, D * HDIM // 128], F8, isOutput=False)
    wp = nc.declare_dram_parameter("wp", [128, PAIRS * D], F8, isOutput=False)
    ident_d = nc.declare_dram_parameter("ident_c", [128, 128], BF16, isOutput=False)
    pT = nc.declare_dram_parameter("pT", [D, N], BF16, isOutput=True)

    with tile.TileContext(nc) as tc:
        with (
            tc.tile_pool(name="big", bufs=1) as big,
            tc.tile_pool(name="exps", bufs=6) as exps,
            tc.tile_pool(name="evac", bufs=4) as evac,
            tc.tile_pool(name="psum", bufs=1, space="PSUM") as psum,
        ):
            # ---- input loads: x chunks + wk first (kT p0 is the critical
            # path to the first scores), then wq, then the rest.
            xts = big.tile([128, KC, N], F8, tag="xts")
            xT_r = xT.rearrange("(c p) n -> p c n", p=128)
            wk_sb = big.tile([128, KC // 2, 2, HDIM], F8, tag="wk")
            wq_sb = big.tile([128, KC // 2, 2, HDIM], F8, tag="wq")
            wv_sb = big.tile([128, KC // 2, 2, HDIM], F8, tag="wv")
            wp_sb = big.tile([128, PAIRS, D], F8, tag="wp")
            ident = big.tile([128, 128], BF16, tag="ident")

            # x arrives in TOKEN slices: the first 512 tokens cover both
            # kT(jt 0-3) and qT(it0) for the whole feature contraction, so
            # the first exp fires after ~2 DMAs instead of the full 2MB of x.
            nc.scalar.dma_start(
                out=wk_sb, in_=wk.rearrange("p (c two w) -> p c two w", two=2, w=HDIM)
            )
            nc.sync.dma_start(out=xts[:, :, 0:512], in_=xT_r[:, :, 0:512])
            nc.scalar.dma_start(
                out=wq_sb, in_=wq.rearrange("p (c two w) -> p c two w", two=2, w=HDIM)
            )
            nc.sync.dma_start(out=xts[:, :, 512:1024], in_=xT_r[:, :, 512:1024])
            nc.scalar.dma_start(
                out=wv_sb, in_=wv.rearrange("p (c two w) -> p c two w", two=2, w=HDIM)
            )
            nc.sync.dma_start(out=xts[:, :, 1024:1536], in_=xT_r[:, :, 1024:1536])
            nc.sync.dma_start(out=xts[:, :, 1536:2048], in_=xT_r[:, :, 1536:2048])
            nc.scalar.dma_start(
                out=wp_sb, in_=wp.rearrange("p (c d) -> p c d", c=PAIRS)
            )
            nc.sync.dma_start(out=ident, in_=ident_d[:, :])

            # v_aug[j, jt, head, 0:64] = v, [..., 64] = 1.0 (denominator trick)
            v_aug = big.tile([128, JT, HPC, 65], BF16, tag="v")
            nc.vector.memset(v_aug[:, :, :, 64:65], 1.0)

            e_set0 = big.tile([128, JT, 1024], BF16, tag="eset0")
            e_set1 = big.tile([128, JT, 1024], BF16, tag="eset1")
            e_sets = [e_set0, e_set1]
            qT = big.tile([128, PAIRS, N], BF16, tag="qT")
            kT = big.tile([128, PAIRS, N], BF16, tag="kT")
            outT = big.tile([128, PAIRS, N], F8, tag="outT")

            warm = evac.tile([1, 1], F32, tag="warm")
            nc.scalar.activation(warm, ident[0:1, 0:1], AF.Exp)

            def emit_qk_tile(p, w_sb, dst, nt):
                ps = psum.tile([128, 512], F32, tag="mm", bufs=2)
                for c in range(KC // 2):
                    nc.tensor.matmul(
                        ps,
                        lhsT=w_sb[:, c, :, p * 128:(p + 1) * 128],
                        rhs=xts[:, 2 * c:2 * c + 2, nt * 512:(nt + 1) * 512],
                        start=(c == 0),
                        stop=(c == KC // 2 - 1),
                        perf_mode=mybir.MatmulPerfMode.DoubleRow,
                    )
                nc.vector.tensor_copy(dst[:, p, nt * 512:(nt + 1) * 512], ps)

            def emit_v_chunk(t):
                ps = psum.tile([128, 512], F32, tag="mm", bufs=2)
                for c in range(KC // 2):
                    nc.tensor.matmul(
                        ps[:, 0:HDIM],
                        lhsT=xts[:, 2 * c:2 * c + 2, t * 128:(t + 1) * 128],
                        rhs=wv_sb[:, c, :, :],
                        start=(c == 0),
                        stop=(c == KC // 2 - 1),
                        perf_mode=mybir.MatmulPerfMode.DoubleRow,
                    )
                nc.vector.tensor_copy(
                    v_aug[:, t, :, 0:64],
                    ps[:, 0:HDIM].rearrange("p (h c) -> p h c", c=64),
                )

            def emit_scores(p, it, jt):
                sc = psum.tile([128, 1024], F32, tag="sc", bufs=2)
                for h in range(2):
                    nc.tensor.matmul(
                        sc[:, h * 512:(h + 1) * 512],
                        lhsT=kT[h * 64:(h + 1) * 64, p, jt * 128:(jt + 1) * 128],
                        rhs=qT[h * 64:(h + 1) * 64, p, it * 512:(it + 1) * 512],
                        start=True,
                        stop=True,
                    )
                return sc

            def emit_pv_group(p, g, eset, pvt):
                # one accumulation group (h, ic): 16 back-to-back matmuls over
                # the j-chunks -- exactly one open group per PSUM bank.
                h, ic = divmod(g, 4)
                for jt in range(JT):
                    nc.tensor.matmul(
                        pvt[:, h * 512 + ic * 65:h * 512 + ic * 65 + 65],
                        lhsT=eset[:, jt, h * 512 + ic * 128:h * 512 + (ic + 1) * 128],
                        rhs=v_aug[:, jt, 2 * p + h, :],
                        start=(jt == 0),
                        stop=(jt == JT - 1),
                    )

            def emit_norm_transpose(p, it, pvt):
                # All pvt readers (recip + muls) come first so the next
                # window's PV (WAR on the single pv buffer) unblocks early;
                # transposes land in mm-pool scratch, then evac to outT.
                rc = evac.tile([128, 8], F32, tag="rc", bufs=2)
                dg = evac.tile([128, 8], F32, tag="dg", bufs=2)
                nc.vector.tensor_copy(
                    dg[:, 0:4],
                    pvt[:, 0:260].rearrange("p (g c) -> p g c", c=65)[:, :, 64],
                )
                nc.vector.tensor_copy(
                    dg[:, 4:8],
                    pvt[:, 512:772].rearrange("p (g c) -> p g c", c=65)[:, :, 64],
                )
                nc.vector.reciprocal(rc, dg)
                ois = []
                for ic in range(4):
                    oi = evac.tile([128, 128], BF16, tag="outI", bufs=4)
                    nc.vector.tensor_scalar_mul(
                        oi[:, 0:64],
                        pvt[:, ic * 65:ic * 65 + 64],
                        rc[:, ic:ic + 1],
                    )
                    nc.vector.tensor_scalar_mul(
                        oi[:, 64:128],
                        pvt[:, 512 + ic * 65:512 + ic * 65 + 64],
                        rc[:, 4 + ic:4 + ic + 1],
                    )
                    ois.append(oi)
                ps = psum.tile([128, 512], F32, tag="mm", bufs=2)
                for ic in range(4):
                    nc.tensor.matmul(
                        ps[:, ic * 128:(ic + 1) * 128],
                        lhsT=ois[ic], rhs=ident, start=True, stop=True,
                    )
                for ic in range(4):
                    nc.vector.tensor_copy(
                        outT[:, p, it * 512 + ic * 128:it * 512 + (ic + 1) * 128],
                        ps[:, ic * 128:(ic + 1) * 128],
                    )

            def emit_proj_tile(it, ot, slot=None):
                if slot is not None:
                    pj = slot
                else:
                    pj = psum.tile([128, 512], F32, tag="mm", bufs=2)
                nc.tensor.matmul(
                    pj,
                    lhsT=wp_sb[:, :, ot * 128:(ot + 1) * 128],
                    rhs=outT[:, :, it * 512:(it + 1) * 512],
                    start=True,
                    stop=True,
                    perf_mode=mybir.MatmulPerfMode.DoubleRow,
                )
                o_sb = evac.tile([128, 512], BF16, tag="osb", bufs=8)
                if slot is not None and ot % 2 == 1:
                    nc.scalar.copy(o_sb, pj)
                else:
                    nc.vector.tensor_copy(o_sb, pj)
                deng = nc.scalar if (slot is not None and ot % 2 == 0) else nc.sync
                deng.dma_start(
                    out=pT[ot * 128:(ot + 1) * 128, it * 512:(it + 1) * 512],
                    in_=o_sb,
                )

            # ---- prelude: kT p0 chunk for jt 0-3, qT p0 it0 ----
            emit_qk_tile(0, wk_sb, kT, 0)
            emit_qk_tile(0, wq_sb, qT, 0)

            # filler work queue: (kind, pair, arg), ordered by first use.
            fillers = [
                ("k", 0, 1), ("q", 0, 1), ("k", 0, 2), ("k", 0, 3),
                ("k", 1, 0), ("q", 0, 2), ("k", 1, 1), ("q", 1, 0),
                ("k", 1, 2), ("k", 1, 3), ("q", 0, 3), ("q", 1, 1),
                ("q", 1, 2), ("q", 1, 3),
            ]
            proj_q = []

            def pop_filler():
                if fillers:
                    kind, p_, arg = fillers.pop(0)
                    emit_qk_tile(p_, wk_sb if kind == "k" else wq_sb,
                                 kT if kind == "k" else qT, arg)
                    return True
                if proj_q:
                    it_t, ot = proj_q.pop(0)
                    emit_proj_tile(it_t, ot)
                    return True
                return False

            # Flattened step stream over all (pair, i-tile, j-chunk) steps.
            # Scores are emitted one step ahead (across window boundaries).
            # exp(w, jt) fills e_sets[w % 2]; the PV of window w-1 runs as 8
            # sequential accumulation groups (one per PSUM bank at a time)
            # spread over the first steps of window w, followed by its
            # normalize/transpose chain and projection availability.
            windows = [(p, it) for p in range(PAIRS) for it in range(IT)]
            NW = len(windows)
            steps = NW * JT

            def step_scores(s):
                w, jt = divmod(s, JT)
                p, it = windows[w]
                return emit_scores(p, it, jt)

            prev = None          # (p, it, pvt, eset) of the previous window
            pvt = None
            sc_cur = step_scores(0)
            for s in range(steps):
                w, jt = divmod(s, JT)
                p, it = windows[w]
                eset = e_sets[w % 2]
                if jt == 0:
                    pvt = psum.tile([128, 1024], F32, tag="pv", bufs=1)
                sc = sc_cur
                sc_cur = step_scores(s + 1) if s + 1 < steps else None
                # fillers / v chunks
                if w == 0:
                    emit_v_chunk(jt)
                    if jt in (2, 4, 6, 10):
                        pop_filler()
                elif jt % 2 == 1:
                    pop_filler()
                    if proj_q and jt % 4 == 1:
                        pop_filler()
                nc.scalar.activation(eset[:, jt, :], sc, AF.Exp, scale=SCALE_DEV)
                if w == NW - 1 and jt >= 11:
                    # catch-up: groups (h,ic0) of the last window start once
                    # the previous window's normalize has released the pv
                    # banks, so the drain begins at norm(ic0) directly
                    j2s = range(12) if jt == 11 else [jt]
                    for h in range(2):
                        for j2 in j2s:
                            nc.tensor.matmul(
                                pvt[:, h * 512:h * 512 + 65],
                                lhsT=eset[:, j2, h * 512:h * 512 + 128],
                                rhs=v_aug[:, j2, 2 * p + h, :],
                                start=(j2 == 0),
                                stop=(jt == JT - 1 and j2 == jt),
                            )
                if prev is not None:
                    if jt < 8:
                        emit_pv_group(prev[0], jt, prev[3], prev[2])
                    elif jt == 8:
                        p0, it0, pvt0, _ = prev
                        emit_norm_transpose(p0, it0, pvt0)
                        if p0 == 1:
                            proj_q.extend((it0, ot) for ot in range(D // 128))
                if jt == JT - 1:
                    prev = (p, it, pvt, eset)

            # ---- drain: last window's remaining PV groups + per-ic
            # normalize pipeline + projection (ACT helps with evacs) ----
            p0, it0, pvt0, eset0 = prev
            while fillers:
                pop_filler()
            # leftover mid-stream proj tiles (outT for them is ready)
            for it_t, ot in proj_q:
                emit_proj_tile(it_t, ot)
            proj_q = []
            tmm = psum.tile([128, 512], F32, tag="mm", bufs=2)

            def norm_ic(ic):
                rc = evac.tile([128, 2], F32, tag="rcd", bufs=4)
                nc.vector.reciprocal(
                    rc[:, 0:1], pvt0[:, ic * 65 + 64:ic * 65 + 65]
                )
                nc.vector.reciprocal(
                    rc[:, 1:2], pvt0[:, 512 + ic * 65 + 64:512 + ic * 65 + 65]
                )
                oi = evac.tile([128, 128], BF16, tag="outI", bufs=4)
                nc.vector.tensor_scalar_mul(
                    oi[:, 0:64], pvt0[:, ic * 65:ic * 65 + 64], rc[:, 0:1]
                )
                nc.vector.tensor_scalar_mul(
                    oi[:, 64:128],
                    pvt0[:, 512 + ic * 65:512 + ic * 65 + 64], rc[:, 1:2]
                )
                nc.tensor.matmul(
                    tmm[:, ic * 128:(ic + 1) * 128],
                    lhsT=oi, rhs=ident, start=True, stop=True,
                )
                nc.vector.tensor_copy(
                    outT[:, p0, it0 * 512 + ic * 128:it0 * 512 + (ic + 1) * 128],
                    tmm[:, ic * 128:(ic + 1) * 128],
                )

            # interleave: group burst ic+1 || norm chain ic || proj slice
            # (ot, ic-1).  proj is done per 128-i chunk so each norm wave
            # immediately feeds its projection; evacs alternate DVE/ACT and
            # each ot's [128, 512] staging row is DMA'd once complete.
            sct0 = psum.tile([128, 1024], F32, tag="sc", bufs=2)
            sct1 = psum.tile([128, 1024], F32, tag="sc", bufs=2)
            pj_slot = [sct0[:, 0:512], sct0[:, 512:1024],
                       sct1[:, 0:512], sct1[:, 512:1024],
                       tmm[:, 0:512]][:4]
            osbs = []
            for _oi in range(8):
                osb_d = evac.tile([128, 512], BF16, tag="osb", bufs=8, name=f"osbd{_oi}")
                osbs.append(osb_d)

            for ic in range(4):
                norm_ic(ic)
                if ic + 1 < 4:
                    emit_pv_group(p0, ic + 1, eset0, pvt0)
                    emit_pv_group(p0, 4 + ic + 1, eset0, pvt0)
                for ot in range(8):
                    slot = pj_slot[ot % 4]
                    out_ps = slot[:, (ot // 4) * 256 + (ic % 2) * 128:
                                  (ot // 4) * 256 + (ic % 2) * 128 + 128]
                    nc.tensor.matmul(
                        out_ps,
                        lhsT=wp_sb[:, :, ot * 128:(ot + 1) * 128],
                        rhs=outT[:, :, it0 * 512 + ic * 128:
                                 it0 * 512 + (ic + 1) * 128],
                        start=True, stop=True,
                        perf_mode=mybir.MatmulPerfMode.DoubleRow,
                    )
                    ceng = nc.scalar if ot % 2 else nc.vector
                    ceng_copy = ceng.copy if ot % 2 else ceng.tensor_copy
                    ceng_copy(osbs[ot][:, ic * 128:(ic + 1) * 128], out_ps)
                    if ic == 3:
                        deng = nc.scalar if ot % 2 else nc.sync
                        deng.dma_start(
                            out=pT[ot * 128:(ot + 1) * 128,
                                   it0 * 512:(it0 + 1) * 512],
                            in_=osbs[ot],
                        )
            proj_q = []
    return nc


_NC = None


def _get_nc():
    global _NC
    if _NC is None:
        _NC = build_bass()
        _NC.finalize()
    return _NC


_IDENT = np.eye(128, dtype=NP_BF16)


def _w_dr_layout(w):
    # [D, HDIM] -> [128, KC/2 * 2 * HDIM]: row p, flat (c, plane, m) with
    # plane = DoubleRow pair index; element = w[(2c + plane)*128 + p, m]
    r = w.reshape(KC // 2, 2, 128, HDIM).transpose(2, 0, 1, 3)
    return np.ascontiguousarray(r.reshape(128, -1))


def make_in_maps(x, w_qkv, w_proj):
    x = np.asarray(x, np.float32)
    w_qkv = (np.asarray(w_qkv, np.float32) * WSCALE).astype(NP_F8)
    w_proj = (np.asarray(w_proj, np.float32) * WSCALE).astype(NP_F8)
    xTs = [np.ascontiguousarray(x[b].T.astype(NP_F8)) for b in range(B)]
    in_maps = []
    for c in range(N_CORES):
        b, g = divmod(c, TP)
        h0 = g * HDIM
        wp_r = w_proj[h0:h0 + HDIM, :].reshape(PAIRS, 128, D).transpose(1, 0, 2)
        in_maps.append({
            "xT": xTs[b],
            "wq": _w_dr_layout(w_qkv[:, h0:h0 + HDIM]),
            "wk": _w_dr_layout(w_qkv[:, D + h0:D + h0 + HDIM]),
            "wv": _w_dr_layout(w_qkv[:, 2 * D + h0:2 * D + h0 + HDIM]),
            "wp": np.ascontiguousarray(wp_r.reshape(128, -1)),
            "ident_c": _IDENT,
        })
    return in_maps


def combine_outputs(x, results):
    x = np.asarray(x, np.float32)
    out = np.empty((B, N, D), np.float32)
    for b in range(B):
        acc = x[b].astype(np.float64)
        for g in range(TP):
            acc += results[b * TP + g]["pT"].astype(np.float32).T / (WSCALE * WSCALE)
        out[b] = acc.astype(np.float32)
    return out


def kernel(x, w_qkv, w_proj):
    nc = _get_nc()
    in_maps = make_in_maps(x, w_qkv, w_proj)
    res = run_bass_kernel_spmd(nc, in_maps, list(range(N_CORES))).results
    return combine_outputs(x, res)
